# revision 20
# baseline (speedup 1.0000x reference)
"""Correlation-layer kernel for Trainium2 (8 NeuronCores, data-parallel over batch).

Problem (per batch b):
    corr[k, m] = sum_c x[b, c, u, v] * y[b, c, i, j],  k = v*h+u, m = i*w+j
    out = relu(corr) / sqrt(sum_k relu(corr)^2 + eps)   (normalize over k per m)

Shapes: x, y = (8, 128, 48, 64) fp32 -> out (8, 3072, 48, 64) fp32.
Sharding: 1 batch per core.

Layout: m on partitions, k on the free dim (normalization over k becomes a
free-dim reduce + per-partition scale; output DMA'd as [M, K] fp16 and
transposed/upcast on the host).

Per m-tile (128 m x 3072 k):
  - 6 fp16 matmuls -> two 3-bank psum tiles.
  - relu evac psum -> fp16 r16: ACT takes psA + psB[:EV_B_ACT], DVE the rest.
  - ss = sum_k relu^2: DVE scalar_tensor_tensor (max0*x, accum_out) on the
    leading cols + ACT Square (accum_out) on the trailing SS_ACT cols.
  - 1/sqrt batched across GR tiles: one ACT Sqrt + one DVE reciprocal on
    [128, GR].
  - out16 = r16 * s: one DVE tensor_scalar in 4x mode.
  - one contiguous 786 KB HWDGE DMA per m-tile.
"""

import sys

sys.path.insert(0, "/opt/trn_rl_repo")

import numpy as np

_BUILD_CACHE = {}

B, C, H, W = 8, 128, 48, 64
K = W * H      # 3072 output channels, k = v*h+u
M = H * W      # 3072 spatial positions, m = i*w+j
NT = M // 128  # 24 m-tiles
EPS = 1e-6

PSA = 1536     # cols in first psum tile
PSB = 1536     # cols in second psum tile
EV_B_ACT = 1024  # leading cols of psB evac'd by ACT (bank-aligned), rest DVE
SS_ACT = 832   # trailing cols of the square+accum pass on ACT Square
GR = 4         # rsqrt-chain batching group (tiles per sqrt/recip instr)
WARMUP_MM = 16  # junk matmuls at start to ramp the PE HAM clock
A_CHUNKS = 3   # split the a-matrix input load for earlier matmul start


def build():
    from concourse import bacc, bass, mybir, tile

    F32 = mybir.dt.float32
    F16 = mybir.dt.float16
    AF = mybir.ActivationFunctionType
    OP = mybir.AluOpType

    nc = bacc.Bacc("TRN2", debug=False, target_bir_lowering=False)

    a_d = nc.dram_tensor("a", [C, K], F16, kind="ExternalInput")
    b_d = nc.dram_tensor("b", [C, M], F16, kind="ExternalInput")
    out_d = nc.dram_tensor("out", [M, K], F16, kind="ExternalOutput")
    junk_d = nc.dram_tensor("junkout", [128, 512], F32, kind="ExternalOutput")

    with tile.TileContext(nc) as tc:
        with (
            tc.tile_pool(name="pers", bufs=1) as pers,
            tc.tile_pool(name="rk", bufs=GR + 2) as rk,
            tc.tile_pool(name="sq", bufs=3) as sqp,
            tc.tile_pool(name="ok", bufs=3) as ok,
            tc.tile_pool(name="sm", bufs=2) as sm,
            tc.tile_pool(name="ps", bufs=1, space=bass.MemorySpace.PSUM) as ps,
        ):
            a_t = pers.tile([C, K], F16)
            b_t = pers.tile([C, M], F16)
            nc.sync.dma_start(b_t[:], b_d[:])
            ac = K // A_CHUNKS
            for j in range(A_CHUNKS):
                nc.scalar.dma_start(
                    a_t[:, j * ac : (j + 1) * ac], a_d[:, j * ac : (j + 1) * ac]
                )

            junk_ps = ps.tile([128, 512], F32, tag="junk")

            def jmm(n):
                for _ in range(n):
                    nc.tensor.matmul(
                        junk_ps[:, 0:128], b_t[:, 0:128], a_t[:, 0:128],
                        start=True, stop=True, skip_group_check=True,
                    )

            state = {}
            groups = {}

            def emit_front(i):
                m0 = i * 128
                psA = ps.tile([128, PSA], F32, tag="psA")
                psB = ps.tile([128, PSB], F32, tag="psB")
                for j in range(PSA // 512):
                    nc.tensor.matmul(
                        psA[:, j * 512 : (j + 1) * 512],
                        b_t[:, m0 : m0 + 128],
                        a_t[:, j * 512 : (j + 1) * 512],
                        start=True, stop=True,
                    )
                for j in range(PSB // 512):
                    nc.tensor.matmul(
                        psB[:, j * 512 : (j + 1) * 512],
                        b_t[:, m0 : m0 + 128],
                        a_t[:, PSA + j * 512 : PSA + (j + 1) * 512],
                        start=True, stop=True,
                    )
                r16 = rk.tile([128, K], F16, tag="r16")
                nc.scalar.activation(r16[:, 0:PSA], psA[:], AF.Relu)
                if EV_B_ACT:
                    nc.scalar.activation(
                        r16[:, PSA : PSA + EV_B_ACT], psB[:, 0:EV_B_ACT], AF.Relu
                    )
                if EV_B_ACT < PSB:
                    nc.vector.tensor_scalar_max(
                        r16[:, PSA + EV_B_ACT : K], psB[:, EV_B_ACT : PSB], 0.0
                    )
                # group tile holding GR per-tile ss / s columns
                g, j = divmod(i, GR)
                if j == 0:
                    ssg = sm.tile([128, GR], F32, tag="ssg", name=f"ssg{g}")
                    s1g = sm.tile([128, GR], F32, tag="s1g", name=f"s1g{g}")
                    groups[g] = (ssg, s1g)
                ssg, s1g = groups[g]
                sq16 = sqp.tile([128, K], F16, tag="sq16")
                ssd = sm.tile([128, 1], F32, tag="ssd")
                w0 = K - SS_ACT
                nc.vector.scalar_tensor_tensor(
                    out=sq16[:, 0:w0], in0=r16[:, 0:w0], scalar=0.0,
                    in1=r16[:, 0:w0], op0=OP.max, op1=OP.mult, accum_out=ssd[:],
                )
                if SS_ACT:
                    ssa = sm.tile([128, 1], F32, tag="ssa")
                    nc.scalar.activation(
                        sq16[:, w0:K], r16[:, w0:K], AF.Square, accum_out=ssa[:]
                    )
                    nc.vector.scalar_tensor_tensor(
                        out=ssg[:, j : j + 1], in0=ssd[:], scalar=EPS, in1=ssa[:],
                        op0=OP.add, op1=OP.add,
                    )
                else:
                    nc.vector.tensor_scalar_add(ssg[:, j : j + 1], ssd[:], EPS)
                state[i] = r16

            def emit_chain(g):
                """sqrt + reciprocal for a whole group of GR tiles."""
                ssg, s1g = groups[g]
                s0g = sm.tile([128, GR], F32, tag="s0g")
                nc.scalar.activation(s0g[:], ssg[:], AF.Sqrt)
                nc.vector.reciprocal(s1g[:], s0g[:])

            def emit_back(i):
                r16 = state.pop(i)
                g, j = divmod(i, GR)
                _, s1g = groups[g]
                out16 = ok.tile([128, K], F16, tag="out16")
                nc.vector.tensor_scalar(
                    out=out16[:], in0=r16[:], scalar1=s1g[:, j : j + 1],
                    scalar2=None, op0=OP.mult,
                )
                nc.sync.dma_start(out_d[i * 128 : (i + 1) * 128, :], out16[:])

            jmm(WARMUP_MM)
            # software pipeline: back(i) runs GR tiles behind front(i)
            for i in range(NT + GR):
                if i < NT:
                    emit_front(i)
                    if i % GR == GR - 1:
                        emit_chain(i // GR)
                if i >= GR:
                    emit_back(i - GR)

            junk_sb = sm.tile([128, 512], F32, tag="junksb")
            nc.scalar.activation(junk_sb[:], junk_ps[:], AF.Relu)
            nc.sync.dma_start(junk_d[:], junk_sb[:])

    nc.compile()
    return nc


def get_built():
    if "nc" not in _BUILD_CACHE:
        _BUILD_CACHE["nc"] = build()
    return _BUILD_CACHE["nc"]


def make_in_maps(x, y):
    in_maps = []
    for bi in range(B):
        a = np.ascontiguousarray(
            np.asarray(x)[bi].transpose(0, 2, 1).reshape(C, K)
        ).astype(np.float16)
        bm = np.asarray(y)[bi].reshape(C, M).astype(np.float16)
        in_maps.append({"a": a, "b": bm})
    return in_maps


def run(x, y, trace=False):
    from concourse import bass_utils

    nc = get_built()
    in_maps = make_in_maps(x, y)
    res = bass_utils.run_bass_kernel_spmd(
        nc, in_maps, core_ids=list(range(B)), trace=trace
    )
    out = np.empty((B, K, M), dtype=np.float32)
    for bi in range(B):
        out[bi] = res.results[bi]["out"].T  # [M,K] fp16 -> [K,M] fp32
    return out.reshape(B, K, H, W), res


def kernel(x, y):
    out, _ = run(x, y, trace=False)
    return out


# revision 26
# speedup vs baseline: 1.0195x; 1.0195x over previous
"""Correlation-layer kernel for Trainium2 (8 NeuronCores, data-parallel over batch).

Problem (per batch b):
    corr[k, m] = sum_c x[b, c, u, v] * y[b, c, i, j],  k = v*h+u, m = i*w+j
    out = relu(corr) / sqrt(sum_k relu(corr)^2 + eps)   (normalize over k per m)

Shapes: x, y = (8, 128, 48, 64) fp32 -> out (8, 3072, 48, 64) fp32.
Sharding: 1 batch per core.

Layout: m on partitions, k on the free dim (normalization over k becomes a
free-dim reduce + per-partition scale; output DMA'd as [M, K] fp16 and
transposed/upcast on the host).

Per m-tile (128 m x 3072 k):
  - 6 fp16 matmuls -> two 3-bank psum tiles.
  - relu evac psum -> fp16 r16: ACT takes psA + psB[:EV_B_ACT], DVE the rest.
  - ss = sum_k relu^2: DVE scalar_tensor_tensor (max0*x, accum_out) on the
    leading cols + ACT Square (accum_out) on the trailing SS_ACT cols.
  - 1/sqrt batched across GR tiles: one ACT Sqrt + one DVE reciprocal on
    [128, GR].
  - out16 = r16 * s: one DVE tensor_scalar in 4x mode.
  - one contiguous 786 KB HWDGE DMA per m-tile.
"""

import sys

sys.path.insert(0, "/opt/trn_rl_repo")

import numpy as np

_BUILD_CACHE = {}

B, C, H, W = 8, 128, 48, 64
K = W * H      # 3072 output channels, k = v*h+u
M = H * W      # 3072 spatial positions, m = i*w+j
NT = M // 128  # 24 m-tiles
EPS = 1e-6

PSA = 1536     # cols in first psum tile
PSB = 1536     # cols in second psum tile
EV_B_ACT = 1024  # leading cols of psB evac'd by ACT (bank-aligned), rest DVE
SS_ACT = 992   # trailing cols of the square+accum pass on ACT Square
GR = 4         # rsqrt-chain batching group (tiles per sqrt/recip instr)
WARMUP_MM = 8  # junk matmuls at start to ramp the PE HAM clock
A_CHUNKS = 3   # split the a-matrix input load for earlier matmul start
B_CHUNKS = 3   # split the b-matrix input load for earlier matmul start


def build():
    from concourse import bacc, bass, mybir, tile

    F32 = mybir.dt.float32
    F16 = mybir.dt.float16
    AF = mybir.ActivationFunctionType
    OP = mybir.AluOpType

    nc = bacc.Bacc("TRN2", debug=False, target_bir_lowering=False)

    a_d = nc.dram_tensor("a", [C, K], F16, kind="ExternalInput")
    b_d = nc.dram_tensor("b", [C, M], F16, kind="ExternalInput")
    out_d = nc.dram_tensor("out", [M, K], F16, kind="ExternalOutput")
    junk_d = nc.dram_tensor("junkout", [128, 128], F32, kind="ExternalOutput")

    with tile.TileContext(nc) as tc:
        with (
            tc.tile_pool(name="pers", bufs=1) as pers,
            tc.tile_pool(name="rk", bufs=GR + 2) as rk,
            tc.tile_pool(name="sq", bufs=3) as sqp,
            tc.tile_pool(name="ok", bufs=3) as ok,
            tc.tile_pool(name="sm", bufs=2) as sm,
            tc.tile_pool(name="ps", bufs=1, space=bass.MemorySpace.PSUM) as ps,
        ):
            a_t = pers.tile([C, K], F16)
            b_t = pers.tile([C, M], F16)
            bc = M // B_CHUNKS
            ac = K // A_CHUNKS
            for j in range(max(A_CHUNKS, B_CHUNKS)):
                if j < B_CHUNKS:
                    nc.sync.dma_start(
                        b_t[:, j * bc : (j + 1) * bc], b_d[:, j * bc : (j + 1) * bc]
                    )
                if j < A_CHUNKS:
                    nc.scalar.dma_start(
                        a_t[:, j * ac : (j + 1) * ac], a_d[:, j * ac : (j + 1) * ac]
                    )

            junk_ps = ps.tile([128, 512], F32, tag="junk")

            def jmm(n):
                for _ in range(n):
                    nc.tensor.matmul(
                        junk_ps[:, 0:128], b_t[:, 0:128], a_t[:, 0:128],
                        start=True, stop=True, skip_group_check=True,
                    )

            state = {}
            groups = {}

            def emit_front(i):
                m0 = i * 128
                psA = ps.tile([128, PSA], F32, tag="psA")
                psB = ps.tile([128, PSB], F32, tag="psB")
                for j in range(PSA // 512):
                    nc.tensor.matmul(
                        psA[:, j * 512 : (j + 1) * 512],
                        b_t[:, m0 : m0 + 128],
                        a_t[:, j * 512 : (j + 1) * 512],
                        start=True, stop=True,
                    )
                for j in range(PSB // 512):
                    nc.tensor.matmul(
                        psB[:, j * 512 : (j + 1) * 512],
                        b_t[:, m0 : m0 + 128],
                        a_t[:, PSA + j * 512 : PSA + (j + 1) * 512],
                        start=True, stop=True,
                    )
                r16 = rk.tile([128, K], F16, tag="r16")
                nc.scalar.activation(r16[:, 0:PSA], psA[:], AF.Relu)
                if EV_B_ACT:
                    nc.scalar.activation(
                        r16[:, PSA : PSA + EV_B_ACT], psB[:, 0:EV_B_ACT], AF.Relu
                    )
                if EV_B_ACT < PSB:
                    nc.vector.tensor_scalar_max(
                        r16[:, PSA + EV_B_ACT : K], psB[:, EV_B_ACT : PSB], 0.0
                    )
                # group tile holding GR per-tile ss / s columns
                g, j = divmod(i, GR)
                if j == 0:
                    ssg = sm.tile([128, GR], F32, tag="ssg", name=f"ssg{g}")
                    s1g = sm.tile([128, GR], F32, tag="s1g", name=f"s1g{g}")
                    groups[g] = (ssg, s1g)
                ssg, s1g = groups[g]
                sq16 = sqp.tile([128, K], F16, tag="sq16")
                ssd = sm.tile([128, 1], F32, tag="ssd")
                w0 = K - SS_ACT
                nc.vector.scalar_tensor_tensor(
                    out=sq16[:, 0:w0], in0=r16[:, 0:w0], scalar=0.0,
                    in1=r16[:, 0:w0], op0=OP.max, op1=OP.mult, accum_out=ssd[:],
                )
                if SS_ACT:
                    ssa = sm.tile([128, 1], F32, tag="ssa")
                    nc.scalar.activation(
                        sq16[:, w0:K], r16[:, w0:K], AF.Square, accum_out=ssa[:]
                    )
                    nc.vector.scalar_tensor_tensor(
                        out=ssg[:, j : j + 1], in0=ssd[:], scalar=EPS, in1=ssa[:],
                        op0=OP.add, op1=OP.add,
                    )
                else:
                    nc.vector.tensor_scalar_add(ssg[:, j : j + 1], ssd[:], EPS)
                state[i] = r16

            def emit_chain(g):
                """sqrt + reciprocal for a whole group of GR tiles."""
                ssg, s1g = groups[g]
                s0g = sm.tile([128, GR], F32, tag="s0g")
                nc.scalar.activation(s0g[:], ssg[:], AF.Sqrt)
                nc.vector.reciprocal(s1g[:], s0g[:])

            def emit_back(i):
                r16 = state.pop(i)
                g, j = divmod(i, GR)
                _, s1g = groups[g]
                out16 = ok.tile([128, K], F16, tag="out16")
                nc.vector.tensor_scalar(
                    out=out16[:], in0=r16[:], scalar1=s1g[:, j : j + 1],
                    scalar2=None, op0=OP.mult,
                )
                nc.sync.dma_start(out_d[i * 128 : (i + 1) * 128, :], out16[:])

            jmm(WARMUP_MM)
            junk_sb = sm.tile([128, 128], F32, tag="junksb")
            nc.scalar.activation(junk_sb[:], junk_ps[:, 0:128], AF.Relu)
            nc.sync.dma_start(junk_d[:], junk_sb[:])
            # software pipeline: back(i) runs GR tiles behind front(i)
            for i in range(NT + GR):
                if i < NT:
                    emit_front(i)
                    if i % GR == GR - 1:
                        emit_chain(i // GR)
                if i >= GR:
                    emit_back(i - GR)

    nc.compile()
    return nc


def get_built():
    if "nc" not in _BUILD_CACHE:
        _BUILD_CACHE["nc"] = build()
    return _BUILD_CACHE["nc"]


def make_in_maps(x, y):
    in_maps = []
    for bi in range(B):
        a = np.ascontiguousarray(
            np.asarray(x)[bi].transpose(0, 2, 1).reshape(C, K)
        ).astype(np.float16)
        bm = np.asarray(y)[bi].reshape(C, M).astype(np.float16)
        in_maps.append({"a": a, "b": bm})
    return in_maps


def run(x, y, trace=False):
    from concourse import bass_utils

    nc = get_built()
    in_maps = make_in_maps(x, y)
    res = bass_utils.run_bass_kernel_spmd(
        nc, in_maps, core_ids=list(range(B)), trace=trace
    )
    out = np.empty((B, K, M), dtype=np.float32)
    for bi in range(B):
        out[bi] = res.results[bi]["out"].T  # [M,K] fp16 -> [K,M] fp32
    return out.reshape(B, K, H, W), res


def kernel(x, y):
    out, _ = run(x, y, trace=False)
    return out
